# revision 12
# baseline (speedup 1.0000x reference)
"""Trainium2 Bass kernel for nn_CMAModel (control-fused memory attention).

Math (reference):
  q  = x @ Wq.T + ctrl @ Wc.T                  [B,T,C]
  kv = [x; fwd_mem; rev_mem]                   [B,S,C], S = T+M+R = 5440
  k  = kv @ Wk.T ; v = kv @ Wv.T
  per head h (D=128): scores = q_h k_h^T / sqrt(D), causal mask on the
  local T block only; w = softmax(scores); out_h = w_loc v_loc + gate_h *
  (w_mem v_mem); gate = sigmoid(q @ Wg.T + bg); y = concat(out_h) @ Wo.T

Sharding (8 cores, SPMD — one program, per-core behavior via input data):
  core = b*4 + g  (b = batch, g = group 0..3).  24 units of (b, head,
  T-half).  Each core runs 3 "slots": slots 0,1 = both halves of a
  "pair" head, slot 2 = one half of a "single" head (shared with the
  neighbor core).  Per batch:
    g=0: pair h0, single (h1, half A)     g=1: pair h2, single (h1, B)
    g=2: pair h3, single (h4, half A)     g=3: pair h5, single (h5... h4, B)

Implementation notes (v2):
  - All PE operands fp16 (better precision than bf16 at these value
    ranges, same PE throughput).
  - Weight-fusion precomputes (gate fused weights, ctrl bias) on HOST.
  - Causal trim: local score tiles only compute t-columns >= tile diag
    start; masking = one static [128,128] diagonal tile for the pair
    head + data-driven threshold / bias-kill columns for the single
    head (uniform SPMD control flow, per-core data).
  - Softmax denominator: running sums on TWO accumulators, alternating
    DVE / Pool per s-tile; two accumulating ones-matmuls at the end.
  - PSUM->SBUF evacuations on Pool; DMA in few large 3D descriptors.
  - PE stream is woven across slot boundaries (next slot's scores
    interleave with the previous slot's tail AV matmuls).
"""

import numpy as np

B, T, C, H, M, R = 2, 2048, 768, 6, 3072, 320
D = C // H          # 128
S = T + M + R       # 5440
P = 128
NT = (S + P - 1) // P          # 43 s-tiles (last has 64 rows)
NLOC = T // P                  # 16 local s-tiles
NCT = C // P                   # 6 feature tiles
THALF = T // 2                 # 1024
NCH = THALF // 512             # 2 chunks of 512 per half
DSCALE = float(D) ** -0.5

# per-batch slot maps: (pair_head, single_head, single_half) per group
GROUP_MAP = [(0, 1, 0), (2, 1, 1), (3, 4, 0), (5, 4, 1)]


def slot_units(g):
    hp, hs, hsh = GROUP_MAP[g]
    return [(hp, 0), (hp, 1), (hs, hsh)]


def _kchunks():
    out = [(0, 128), (128, 384)]
    off = 512
    while off < S:
        w = min(512, S - off)
        out.append((off, w))
        off += w
    return out


KCH = _kchunks()               # 10x512 + 320


def _tile_plan(k):
    """Per-slot attention tile list: (j, col0) with col0 = first needed
    t-column (causal trim).  Local tiles for slot0 (pair half A) are
    j=0..7 trimmed at 128j; slots 1,2: j=0..7 full, j=8..15 trimmed at
    128(j-8).  Memory tiles always full."""
    loc = []
    if k == 0:
        for j in range(8):
            loc.append((j, 128 * j))
    else:
        for j in range(8):
            loc.append((j, 0))
        for j in range(8, NLOC):
            loc.append((j, 128 * (j - 8)))
    mem = [(j, 0) for j in range(NLOC, NT)]
    # interleave mem-first (mem tiles have no DVE mask work; also makes
    # the first tile of each accumulator parity full-width)
    js = []
    while loc or mem:
        if mem:
            js.append(mem.pop(0))
        if loc:
            js.append(loc.pop(0))
    return js


def build_nc(debug=False):
    import concourse.mybir as mybir
    import concourse.tile as tile
    from concourse import bacc

    f32 = mybir.dt.float32
    f32r = mybir.dt.float32r
    f16 = mybir.dt.float16
    AF = mybir.ActivationFunctionType
    OP = mybir.AluOpType

    nc = bacc.Bacc("TRN2", target_bir_lowering=False, debug=False,
                   num_devices=8)

    def mm(psum, lhsT, rhs, start=True, stop=True):
        nc.tensor.matmul(psum, lhsT, rhs, start=start, stop=stop)

    dram = {}
    for name, shape, dt_ in [
        ("kvT", [C, S], f16),            # batch kv, transposed
        ("xqT", [C, 3 * THALF], f16),    # per-slot x columns, transposed
        ("wqT", [C, 3 * P], f16),        # per-slot Wq head-rows, transposed
        ("wkT0", [C, P], f16),           # pair-head Wk rows, transposed
        ("wkT1", [C, P], f16),           # single-head Wk rows, transposed
        ("wvT2", [C, 2 * P], f16),       # [pair | single] Wv rows, transposed
        ("woT", [P, 3 * C], f16),        # per-slot Wo head-cols, transposed
        ("wfT", [C, 3], f16),            # fused gate weight cols (host)
        ("qbs", [P, 3], f32),            # per-slot q bias column (host)
        ("gb3", [1, 3], f32),            # per-slot gate bias (host)
        ("msk2", [P, NLOC * P], f16),    # slot2 diag strip masks (host)
        ("hA01", [P, 1], f32),           # slot2: 0 if half A else 1
        ("bias2", [P, 8], f32),          # slot2 tiles 8..15 exp bias
        ("mskd", [P, P], f16),           # static diagonal mask c >= i
        ("ones_r", [1, P], f16),         # ones row (bcast stationary)
        ("ones_c16", [P, 1], f16),       # f16 ones col (den reduction)
    ]:
        dram[name] = nc.dram_tensor(name, shape, dt_, kind="ExternalInput")
    yp = nc.dram_tensor("yp", [3 * C, THALF], f32, kind="ExternalOutput")

    from contextlib import ExitStack

    with tile.TileContext(nc) as tc, ExitStack() as _ctx:
        consts = _ctx.enter_context(tc.tile_pool(name="consts", bufs=1))

        # ---- K/V + attention caches ----
        kh0 = consts.tile([P, S], f16)
        kh1 = consts.tile([P, S], f16)
        vh = consts.tile([P, NT, 2 * P], f16)

        # ---- constants into SBUF (weights via pool queue, data via sync) ----
        wk0 = consts.tile([P, NCT, P], f16)
        wk1 = consts.tile([P, NCT, P], f16)
        wv2 = consts.tile([P, NCT, 2 * P], f16)
        nc.gpsimd.dma_start(out=wk0[:], in_=dram["wkT0"][:, :].rearrange(
            "(a p) b -> p a b", p=P))
        nc.gpsimd.dma_start(out=wk1[:], in_=dram["wkT1"][:, :].rearrange(
            "(a p) b -> p a b", p=P))
        nc.gpsimd.dma_start(out=wv2[:], in_=dram["wvT2"][:, :].rearrange(
            "(a p) b -> p a b", p=P))
        xq6 = consts.tile([P, 3, NCT, THALF], f16)

        # ---- phase 2: K/V projections into SBUF caches ----
        with tc.tile_pool(name="kvp", bufs=3) as kvp, \
             tc.tile_pool(name="kvps", bufs=1, space="PSUM") as kvps:
            for sc, (off, w) in enumerate(KCH):
                if sc == 7:
                    # xqT transfers ride the sync queue mid-stream: early
                    # kv chunks see no DMA contention, and xq6 still lands
                    # well before the q projection needs it
                    for k in range(3):
                        nc.sync.dma_start(
                            out=xq6[:, k],
                            in_=dram["xqT"][:, k * THALF:
                                            (k + 1) * THALF].rearrange(
                                "(a p) b -> p a b", p=P))
                kv6 = kvp.tile([P, NCT, 512], f16, tag="kv")
                nc.sync.dma_start(
                    out=kv6[:, :, :w],
                    in_=dram["kvT"][:, off:off + w].rearrange(
                        "(a p) s -> p a s", p=P))
                pk0 = kvps.tile([P, 512], f32, tag="k0", bufs=2)
                pk1 = kvps.tile([P, 512], f32, tag="k1", bufs=2)
                subs = []
                o2 = off
                while o2 < off + w:
                    subs.append((o2 - off, min(P, off + w - o2)))
                    o2 += P
                pv = [kvps.tile([P, 2 * P], f32, tag=f"v{si}",
                                name=f"pv{si}", bufs=1)
                      for si in range(len(subs))]
                for ct in range(NCT):
                    mm(pk0[:, :w], wk0[:, ct, :], kv6[:, ct, :w],
                       start=(ct == 0), stop=(ct == NCT - 1))
                    mm(pk1[:, :w], wk1[:, ct, :], kv6[:, ct, :w],
                       start=(ct == 0), stop=(ct == NCT - 1))
                    for si, (so, sw) in enumerate(subs):
                        mm(pv[si][:sw, :], kv6[:, ct, so:so + sw],
                           wv2[:, ct, :],
                           start=(ct == 0), stop=(ct == NCT - 1))
                nc.scalar.copy(kh0[:, off:off + w], pk0[:, :w])
                nc.scalar.copy(kh1[:, off:off + w], pk1[:, :w])
                for si, (so, sw) in enumerate(subs):
                    j = (off + so) // P
                    nc.vector.tensor_copy(out=vh[:sw, j, :],
                                          in_=pv[si][:sw, :])

        # ---- remaining constants (queued behind the kv stream) ----
        wqt = consts.tile([P, NCT, 3 * P], f16)
        nc.gpsimd.dma_start(out=wqt[:], in_=dram["wqT"][:, :].rearrange(
            "(a p) b -> p a b", p=P))
        wft = consts.tile([P, NCT, 3], f16)
        nc.gpsimd.dma_start(out=wft[:], in_=dram["wfT"][:, :].rearrange(
            "(a p) b -> p a b", p=P))
        wot = consts.tile([P, 3 * C], f16)
        nc.gpsimd.dma_start(out=wot[:], in_=dram["woT"][:, :])
        qbs = consts.tile([P, 3], f32)
        nc.gpsimd.dma_start(out=qbs[:], in_=dram["qbs"][:, :])
        gb3 = consts.tile([1, 3], f32)
        nc.gpsimd.dma_start(out=gb3[:], in_=dram["gb3"][:, :])
        msk2 = consts.tile([P, NLOC, P], f16)
        nc.gpsimd.dma_start(out=msk2[:], in_=dram["msk2"][:, :].rearrange(
            "p (a b) -> p a b", a=NLOC))
        hA01 = consts.tile([P, 1], f32)
        nc.gpsimd.dma_start(out=hA01[:], in_=dram["hA01"][:, :])
        bias2 = consts.tile([P, 8], f32)
        nc.gpsimd.dma_start(out=bias2[:], in_=dram["bias2"][:, :])
        mskd = consts.tile([P, P], f16)
        nc.gpsimd.dma_start(out=mskd[:], in_=dram["mskd"][:, :])
        ones_row = consts.tile([1, P], f16)
        nc.sync.dma_start(out=ones_row[:], in_=dram["ones_r"][:, :])
        ones_c16 = consts.tile([P, 1], f16)
        nc.sync.dma_start(out=ones_c16[:], in_=dram["ones_c16"][:, :])

        # ---- phase 3: q projection + gate ----
        qsb = consts.tile([P, 3, THALF], f16)
        gate = consts.tile([1, 3, THALF], f32)
        with tc.tile_pool(name="qps", bufs=1, space="PSUM") as qps:
            for k in range(3):
                for ch in range(NCH):
                    pq = qps.tile([P, 512], f32, tag="q", bufs=2)
                    pg = qps.tile([1, 512], f32, tag="g", bufs=2)
                    for ct in range(NCT):
                        mm(pq[:], wqt[:, ct, k * P:(k + 1) * P],
                           xq6[:, k, ct, ch * 512:(ch + 1) * 512],
                           start=(ct == 0), stop=(ct == NCT - 1))
                        mm(pg[:], wft[:, ct, k:k + 1],
                           xq6[:, k, ct, ch * 512:(ch + 1) * 512],
                           start=(ct == 0), stop=(ct == NCT - 1))
                    nc.vector.tensor_scalar_add(
                        qsb[:, k, ch * 512:(ch + 1) * 512], pq[:],
                        qbs[:, k:k + 1])
                    nc.scalar.activation(
                        gate[0:1, k, ch * 512:(ch + 1) * 512], pg[:],
                        AF.Sigmoid, bias=gb3[0:1, k:k + 1], scale=1.0)

        # ---- phase 4: attention + output projection, woven slot stream ----
        with tc.tile_pool(name="att", bufs=2) as att_pool, \
             tc.tile_pool(name="ep", bufs=10) as ep, \
             tc.tile_pool(name="mp", bufs=4) as mpp, \
             tc.tile_pool(name="vec", bufs=3) as vec, \
             tc.tile_pool(name="cmb", bufs=1) as cmb, \
             tc.tile_pool(name="ysb", bufs=2) as ysb, \
             tc.tile_pool(name="aps", bufs=1, space="PSUM") as aps:

            # global stream of (slot, j, col0)
            stream = []
            slot_first_idx = {}
            slot_last = {}
            for k in range(3):
                plan = _tile_plan(k)
                slot_first_idx[k] = len(stream)
                for (j, col0) in plan:
                    stream.append((k, j, col0))
                # last tile index per region for stop flags
                lmax = max(j for (j, _) in plan if j < NLOC)
                mmax = max(j for (j, _) in plan if j >= NLOC)
                slot_last[k] = (lmax, mmax)

            st = {}  # per-slot state

            def emit_av(k, j, col0, E2):
                spn = min(P, S - j * P)
                s = st[k]
                reg = 'l' if j < NLOC else 'm'
                voff = 0 if k < 2 else P
                first = j == 0 or j == NLOC
                last = j in slot_last[k]
                for ch in range(NCH):
                    g0 = max(512 * ch, col0)
                    g1 = 512 * (ch + 1)
                    if g1 <= g0:
                        continue
                    if first:
                        s['pacc'][(ch, reg)] = aps.tile(
                            [P, 512], f32, tag=f"{reg}{ch}",
                            name=f"p{reg}{ch}")
                    mm(s['pacc'][(ch, reg)][:, g0 - 512 * ch:g1 - 512 * ch],
                       vh[:spn, j, voff:voff + P], E2[:spn, g0:g1],
                       start=first, stop=last)
                if last:
                    # evacuate this region's accumulators (Pool)
                    dst = s['Lsb'] if reg == 'l' else s['Msb']
                    for ch in range(NCH):
                        nc.vector.tensor_copy(
                            out=dst[:, ch, :],
                            in_=s['pacc'].pop((ch, reg))[:])

            def finalize(k):
                # fully per-ch pipelined: ch0's output matmuls/copies/DMA
                # overlap ch1's normalization chain.  The last slot uses
                # per-ot DMAs so the kernel tail is copy/DMA pipelined.
                s = st[k]
                attb = att_pool.tile([P, NCH, 512], f16, tag="attb")
                for ch in range(NCH):
                    cs = slice(ch * 512, (ch + 1) * 512)
                    pden = aps.tile([1, 512], f32, tag="sc", bufs=2)
                    mm(pden[:], ones_c16[:], s['Rt'][:, cs])
                    rr32 = vec.tile([1, 512], f32, tag="rr32")
                    nc.vector.reciprocal_approx_fast(out=rr32[:],
                                                     in_=pden[:])
                    rr = vec.tile([1, 512], f16, tag="rr")
                    gr = vec.tile([1, 512], f16, tag="gr")
                    with nc.allow_low_precision(reason="f16 norm"):
                        nc.vector.tensor_copy(out=rr[:], in_=rr32[:])
                        nc.vector.tensor_tensor(
                            gr[:], gate[0:1, k, cs], rr32[:], OP.mult)
                    prb = aps.tile([P, 512], f32, tag="sc", bufs=2)
                    mm(prb[:], ones_row[:], rr[:])
                    pgb = aps.tile([P, 512], f32, tag="sc", bufs=2)
                    mm(pgb[:], ones_row[:], gr[:])
                    t1 = cmb.tile([P, 512], f32, tag="t1")
                    nc.vector.tensor_tensor(t1[:], s['Lsb'][:, ch, :],
                                            prb[:], OP.mult)
                    t2 = cmb.tile([P, 512], f32, tag="t2")
                    nc.vector.tensor_tensor(t2[:], s['Msb'][:, ch, :],
                                            pgb[:], OP.mult)
                    nc.vector.tensor_tensor(attb[:, ch, :], t1[:],
                                            t2[:], OP.add)
                    yst = ysb.tile([P, NCT, 512], f32, tag="y")
                    for ot in range(NCT):
                        py = aps.tile([P, 512], f32, tag="sc", bufs=2)
                        mm(py[:], wot[:, k * C + ot * P:k * C + (ot + 1) * P],
                           attb[:, ch, :])
                        if ot % 2 == 0:
                            nc.vector.tensor_copy(out=yst[:, ot, :],
                                                  in_=py[:])
                        else:
                            nc.scalar.copy(yst[:, ot, :], py[:])
                        if k == 2:
                            nc.sync.dma_start(
                                out=yp[k * C + ot * P:k * C + (ot + 1) * P,
                                       ch * 512:(ch + 1) * 512],
                                in_=yst[:, ot, :])
                    if k < 2:
                        nc.sync.dma_start(
                            out=yp[k * C:(k + 1) * C,
                                   ch * 512:(ch + 1) * 512].rearrange(
                                       "(a p) b -> p a b", p=P),
                            in_=yst[:])

            pend = []
            fin_queue = []
            for idx, (k, j, col0) in enumerate(stream):
                s = st.get(k)
                if s is None:
                    s = st[k] = {
                        'Rt': vec.tile([P, THALF], f16, tag="Rt",
                                       name=f"Rt{k}"),
                        'Lsb': att_pool.tile([P, NCH, 512], f32, tag="Lsb",
                                             name=f"Lsb{k}"),
                        'Msb': att_pool.tile([P, NCH, 512], f32, tag="Msb",
                                             name=f"Msb{k}"),
                        'pacc': {},
                        'n': 0,
                    }
                if fin_queue and idx == slot_first_idx[k] + 8:
                    finalize(fin_queue.pop(0))
                spn = min(P, S - j * P)
                w = THALF - col0
                kh = kh0 if k < 2 else kh1
                # scores -> psum (chunks of <=512)
                ps = aps.tile([P, NCH, 512], f32, tag="sc", bufs=2)
                psf = ps[:spn].rearrange("p a b -> p (a b)")
                c = 0
                while c < w:
                    cw = min(512, w - c)
                    mm(psf[:, c:c + cw], kh[:, j * P:j * P + spn],
                       qsb[:, k, col0 + c:col0 + c + cw])
                    c += cw
                # exp (+ data-driven kill bias for slot2 tiles 8..15)
                E2 = ep.tile([P, THALF], f16, tag="E")
                if k == 2 and NLOC > j >= 8:
                    nc.scalar.activation(E2[:spn, col0:], psf[:, :w], AF.Exp,
                                         bias=bias2[:spn, j - 8:j - 7],
                                         scale=DSCALE)
                else:
                    nc.scalar.activation(E2[:spn, col0:], psf[:, :w], AF.Exp,
                                         scale=DSCALE)
                # causal masking
                if j < NLOC:
                    if k < 2:
                        if k == 0 or j >= 8:
                            # static diagonal tile on the first 128 cols
                            nc.vector.tensor_tensor(
                                E2[:spn, col0:col0 + P],
                                E2[:spn, col0:col0 + P], mskd[:spn], OP.mult)
                    else:
                        if j < 8 and j > 0:
                            # half-A kills t < 128j entirely
                            nc.vector.tensor_scalar(
                                E2[:spn, 0:128 * j],
                                E2[:spn, 0:128 * j],
                                hA01[:spn], None, OP.mult)
                        # data-driven diagonal strip on the tile's diag cols
                        d0 = 128 * j if j < 8 else col0
                        nc.vector.tensor_tensor(
                            E2[:spn, d0:d0 + P], E2[:spn, d0:d0 + P],
                            msk2[:spn, j, :], OP.mult)
                # running softmax denominator
                acc = s['Rt']
                if s['n'] == 0:
                    nc.vector.tensor_copy(out=acc[:, :], in_=E2[:, :])
                else:
                    nc.vector.tensor_tensor(acc[:spn, col0:],
                                            acc[:spn, col0:],
                                            E2[:spn, col0:], OP.add)
                s['n'] += 1
                pend.append((k, j, col0, E2))
                if len(pend) > 4:
                    emit_av(*pend.pop(0))
                if idx == len(stream) - 1 or stream[idx + 1][0] != k:
                    fin_queue.append(k)
            for item in pend:
                emit_av(*item)
            while fin_queue:
                finalize(fin_queue.pop(0))
    nc.compile()
    return nc


def make_in_maps(x, forward_memory, reverse_memory, ctrl, Wq, Wk, Wv, Wo,
                 Wc, Wg, bg):
    f = np.float32
    h = np.float16
    ii = np.arange(P).reshape(P, 1)
    cc = np.arange(P).reshape(1, P)
    mskd = (cc >= ii).astype(h)
    ones_r = np.ones((1, P), dtype=h)
    ones_c16 = np.ones((P, 1), dtype=h)
    # host-side weight fusion
    qbf = (Wc @ ctrl).astype(f)                    # [C] ctrl @ Wc.T
    wf_full = (Wq.T @ Wg.T).astype(f)              # [C, H]
    gbf = (Wg @ qbf + bg).astype(f)                # [H]
    in_maps = []
    for core in range(8):
        b, g = core // 4, core % 4
        units = slot_units(g)
        hp, hs, hsh = GROUP_MAP[g]
        kv = np.concatenate(
            [x[b], forward_memory[b], reverse_memory[b]], axis=0)
        kvT = np.ascontiguousarray(kv.T, dtype=h)
        xqT = np.concatenate(
            [np.ascontiguousarray(x[b, h2 * THALF:(h2 + 1) * THALF, :].T)
             for (_, h2) in units], axis=1).astype(h)
        wqT = np.concatenate(
            [np.ascontiguousarray(Wq[hh * P:(hh + 1) * P, :].T)
             for (hh, _) in units], axis=1).astype(h)
        wkT0 = np.ascontiguousarray(Wk[hp * P:(hp + 1) * P, :].T).astype(h)
        wkT1 = np.ascontiguousarray(Wk[hs * P:(hs + 1) * P, :].T).astype(h)
        wvT2 = np.concatenate(
            [np.ascontiguousarray(Wv[hh * P:(hh + 1) * P, :].T)
             for hh in (hp, hs)], axis=1).astype(h)
        woT = np.concatenate(
            [np.ascontiguousarray(Wo[:, hh * P:(hh + 1) * P].T)
             for (hh, _) in units], axis=1).astype(h)
        wfT = np.stack([wf_full[:, hh] for (hh, _) in units],
                       axis=1).astype(h)
        qbs = np.stack([qbf[hh * P:(hh + 1) * P] for (hh, _) in units],
                       axis=1).astype(f)
        gb3 = np.array([[gbf[hh] for (hh, _) in units]], dtype=f)
        # slot2 causal data: diag strip masks, half-A kill, bias kill.
        # strip for tile j covers cols [d0, d0+128), d0 = 128j (j<8) or
        # 128(j-8) (j>=8): halfA j<8 -> diagonal; halfB j<8 -> all-ones;
        # halfB j>=8 -> diagonal; halfA j>=8 -> zeros (bias2 kills too)
        msk2 = np.empty((P, NLOC, P), dtype=h)
        diag = (cc >= ii)
        for j in range(NLOC):
            if j < 8:
                msk2[:, j, :] = np.float16(1.0) if hsh else diag.astype(h)
            else:
                msk2[:, j, :] = diag.astype(h) if hsh else np.float16(0.0)
        hA01 = np.full((P, 1), float(hsh), dtype=f)
        bias2 = np.full((P, 8), 0.0 if hsh else -30000.0, dtype=f)
        in_maps.append({
            "kvT": kvT, "xqT": np.ascontiguousarray(xqT),
            "wqT": np.ascontiguousarray(wqT),
            "wkT0": wkT0, "wkT1": wkT1,
            "wvT2": np.ascontiguousarray(wvT2),
            "woT": np.ascontiguousarray(woT),
            "wfT": np.ascontiguousarray(wfT),
            "qbs": np.ascontiguousarray(qbs), "gb3": gb3,
            "msk2": msk2.reshape(P, NLOC * P), "hA01": hA01,
            "bias2": bias2,
            "mskd": mskd, "ones_r": ones_r, "ones_c16": ones_c16,
        })
    return in_maps


def unshard(results):
    y = np.zeros((B, T, C), dtype=np.float32)
    for core in range(8):
        b, g = core // 4, core % 4
        ypc = results[core]["yp"]
        for kslot, (_, half) in enumerate(slot_units(g)):
            y[b, half * THALF:(half + 1) * THALF, :] += \
                ypc[kslot * C:(kslot + 1) * C, :].T
    return y


_nc_cache = {}


def _get_nc(debug=False):
    key = (debug,)
    if key not in _nc_cache:
        _nc_cache[key] = build_nc(debug)
    return _nc_cache[key]


def kernel(**inputs):
    return kernel_ex(**inputs)[0]


def kernel_ex(trace=False, trace_cores=None, use_f32r=True, debug=False,
              att_bf16=True, **inputs):
    from concourse.bass_utils import run_bass_kernel_spmd

    np_inputs = {k: np.asarray(v) for k, v in inputs.items()}
    in_maps = make_in_maps(**np_inputs)
    nc = _get_nc(debug)
    res = run_bass_kernel_spmd(nc, in_maps, list(range(8)), trace=trace,
                               trace_cores=trace_cores)
    return unshard(res.results), res
